# revision 13
# baseline (speedup 1.0000x reference)
"""Trainium2 Bass kernel for CompoundMultivariateEmbedding.

Math: out[n] = concat(level_tab[l], type_tab[t], feat_tab[f], exch_tab[e],
pair_tab[p]) @ W.T + b = P_lvl[l] + P_pair[p] + C0 + t*D1 + f*D2 + e*E1 + e^2*E2
where P_b = tab_b @ W[:, block_b].T.  The 2-row type/feature blocks are linear
in their index and the 3-row exchange block is an exact quadratic, so only
level (50 rows) and pair (20 rows) need one-hot treatment.  C0 (the constant
term + bias) folds into every level row.  The [88, 128] coefficient matrix P
is precomputed host-side in fp16 (quantization error ~5e-3 abs vs a ~0.11 abs
gate); e^2 is precomputed host-side as a sixth int32 index array.

Per-core loop (tokens sharded 8 ways, 131072/core, superbatches of 8192):
  1. six idx int32 -> fp16 SWDGE cast-DMAs (l -> p96, p -> p64 of idxf;
     t/f/e/e2 -> rows 84-87 of the stationary tile st directly)
  2. build idxb rows 0-49 (level) + 64-83 (pair): GPSIMD partition_broadcast
     (32-aligned bases 0/32/64) on bcast superbatches, or a tiny PE matmul
     (e_sel [2,84] stationary at rows 96-97, fp16 PSUM out) on the rest
  3. DVE is_equal vs per-partition iota -> one-hot st[0:84] fp16
     (4x mode from SBUF, 2x mode from PSUM); junk rows 50-63 hit P rows = 0
  4. per 1024 tokens: 8 matmuls, stationary = strided st[0:88] slice
     (token 8p+m -> partition p), moving = P fp16 [88, 128], fp32 PSUM out
  5. ACT copies PSUM -> SBUF staging; HWDGE stores 1 MiB per 2048 tokens
     (two 4 KiB contiguous chunks per partition)
"""

import sys

sys.path.insert(0, "/opt/trn_rl_repo")

import numpy as np

import concourse.bass as bass
import concourse.tile as tile
from concourse import bacc, library_config, mybir
from concourse._compat import with_exitstack

F32 = mybir.dt.float32
F16 = mybir.dt.float16
BF16 = mybir.dt.bfloat16
I32 = mybir.dt.int32

N_FULL = 1048576
N_CORES = 8
EMBED = 128

IDX_NAMES = ["level_idx", "type_idx", "feature_idx", "exchange_idx", "pair_idx"]

VR = 88  # stationary rows: 0-49 level, 50-63 junk, 64-83 pair, 84-87 t/f/e/e2
L0, LN = 0, 50
P0, PN = 64, 20
RAW0 = 84  # t, f, e, e2 rows

SB = 8192  # tokens per superbatch (idx DMA + one-hot build granularity)
TB = 1024  # tokens per PSUM batch (8 matmul tiles)
N_BCAST = 0  # GPSIMD partition_broadcast measured 8% of roofline + wrong: off


@with_exitstack
def _emb_kernel(ctx, tc, y_ap, pt_ap, esel_ap, iota_ap, idxs, n_core, n_bcast):
    nc = tc.nc

    if n_bcast > 0:
        nc.gpsimd.load_library(library_config.proxy)

    const = ctx.enter_context(tc.tile_pool(name="const", bufs=1))

    # ---- constants (host-precomputed) ----
    pt = const.tile([VR, EMBED], F16)  # coefficient matrix
    nc.sync.dma_start(pt, pt_ap)
    iota_col = const.tile([VR, 1], F32)  # within-block row index per partition
    nc.sync.dma_start(iota_col, iota_ap)
    e_sel = const.tile([128, VR], F16)  # level/pair masks at partitions 96-97
    nc.gpsimd.dma_start(e_sel[96:98, :], esel_ap)

    # ---- pools ----
    idx_pool = ctx.enter_context(tc.tile_pool(name="idxp", bufs=2))
    idxb_pool = ctx.enter_context(tc.tile_pool(name="idxbp", bufs=2))
    st_pool = ctx.enter_context(tc.tile_pool(name="stp", bufs=2))
    out_pool = ctx.enter_context(tc.tile_pool(name="outp", bufs=3))
    pbc_pool = ctx.enter_context(
        tc.tile_pool(name="pbc", bufs=3, space=bass.MemorySpace.PSUM)
    )
    pout_pool = ctx.enter_context(
        tc.tile_pool(name="pout", bufs=2, space=bass.MemorySpace.PSUM)
    )

    n_sb = n_core // SB

    def emit_loads(si):
        """Prefetch superbatch si: idx cast-DMAs into fresh idxf/st tiles."""
        s0 = si * SB
        st = st_pool.tile([VR, SB], F16, tag="st")
        idxf = idx_pool.tile([128, SB], F16, tag="idxf")
        nc.gpsimd.dma_start(idxf[96:97, :], idxs[0][s0 : s0 + SB])
        nc.gpsimd.dma_start(idxf[97:98, :], idxs[4][s0 : s0 + SB])
        for k, j in enumerate([1, 2, 3, 5]):
            nc.gpsimd.dma_start(st[RAW0 + k : RAW0 + k + 1, :], idxs[j][s0 : s0 + SB])
        return idxf, st

    def emit_build(idxf, st, h2):
        """One-hot build for tokens [h2*1024, (h2+1)*1024) of a superbatch."""
        psbc = pbc_pool.tile([RAW0, 1024], F32, tag="psbc")
        for g in range(2):
            nc.tensor.matmul(
                psbc[:, bass.ts(g, 512)],
                e_sel[96:98, 0:RAW0],
                idxf[96:98, h2 * 1024 + g * 512 : h2 * 1024 + (g + 1) * 512],
                tile_position=(96, 0),
            )
        nc.vector.tensor_scalar(
            st[0:RAW0, bass.ts(h2, 1024)],
            psbc,
            iota_col[0:RAW0, :],
            None,
            mybir.AluOpType.is_equal,
        )

    def emit_store(si, st, tbp, build_units):
        """Project + store tokens [tbp*2048, (tbp+1)*2048) of superbatch si,
        interleaving next-superbatch one-hot builds between pso units."""
        st_r = st.rearrange("v (t p q) -> v t q p", q=8, p=128)
        osb = out_pool.tile([128, 2 * TB], F32, tag="osb")
        for u in range(4):  # 512-token pso units
            if u % 2 == 0 and build_units:
                build_units.pop(0)()
            tb = tbp * 2 + u // 2
            pso = pout_pool.tile([128, 512], F32, tag="pso")
            for m in range(4):
                mg = (u % 2) * 4 + m
                nc.tensor.matmul(
                    pso[:, bass.ts(m, 128)],
                    st_r[:, tb, mg, :],
                    pt,
                    start=True,
                    stop=True,
                )
            nc.scalar.copy(osb[:, u * 512 : (u + 1) * 512], pso)
        n0 = si * SB + tbp * 2 * TB
        dview = y_ap[n0 : n0 + 2 * TB, :].rearrange(
            "(h p q) e -> p h q e", h=2, p=128, q=8
        )
        nc.sync.dma_start(dview, osb.rearrange("p (h q e) -> p h q e", h=2, q=8))

    # software pipeline: build superbatch sb+1 interleaved with sb's stores
    cur = emit_loads(0)
    for h2 in range(SB // 1024):
        emit_build(cur[0], cur[1], h2)
    for sb in range(n_sb):
        nxt = emit_loads(sb + 1) if sb + 1 < n_sb else None
        builds = (
            [
                (lambda h2=h2: emit_build(nxt[0], nxt[1], h2))
                for h2 in range(SB // 1024)
            ]
            if nxt is not None
            else []
        )
        for tbp in range(SB // (2 * TB)):
            emit_store(sb, cur[1], tbp, builds)
        cur = nxt

def build(n_core, n_bcast=N_BCAST, num_devices=N_CORES):
    nc = bacc.Bacc(
        "TRN2", target_bir_lowering=False, debug=False, num_devices=num_devices
    )
    pt_ap = nc.dram_tensor("ptab", [VR, EMBED], F16, kind="ExternalInput").ap()
    esel_ap = nc.dram_tensor("esel", [2, VR], F16, kind="ExternalInput").ap()
    iota_ap = nc.dram_tensor("iotac", [VR, 1], F32, kind="ExternalInput").ap()
    idxs = []
    for nm in IDX_NAMES + ["e2_idx"]:
        idxs.append(nc.dram_tensor(nm, [n_core], I32, kind="ExternalInput").ap())
    y = nc.dram_tensor("y", [n_core, EMBED], F32, kind="ExternalOutput")

    with tile.TileContext(nc) as tc:
        _emb_kernel(tc, y.ap(), pt_ap, esel_ap, iota_ap, idxs, n_core, n_bcast)
    nc.compile()
    return nc


_NC_CACHE = {}


def _get_nc(n_core, n_bcast=N_BCAST):
    key = (n_core, n_bcast)
    if key not in _NC_CACHE:
        _NC_CACHE[key] = build(n_core, n_bcast)
    return _NC_CACHE[key]


def _make_consts(inputs):
    """Host-side prep of the tiny [88,128] fp16 coefficient matrix + masks."""
    w = np.asarray(inputs["W"], np.float32)
    b = np.asarray(inputs["b"], np.float32)
    tabs = [
        np.asarray(inputs[nm], np.float32)
        for nm in ["level_tab", "type_tab", "feature_tab", "exchange_tab", "pair_tab"]
    ]
    foff = [0, 25, 50, 75, 100]
    pj = [t @ w[:, f : f + t.shape[1]].T for t, f in zip(tabs, foff)]
    c0 = pj[1][0] + pj[2][0] + pj[3][0] + b  # type/feat/exch row 0 + bias
    e2c = (pj[3][2] - 2.0 * pj[3][1] + pj[3][0]) / 2.0
    e1c = pj[3][1] - pj[3][0] - e2c
    p = np.zeros((VR, EMBED), np.float32)
    p[L0 : L0 + LN] = pj[0] + c0[None, :]
    p[P0 : P0 + PN] = pj[4]
    p[RAW0 + 0] = pj[1][1] - pj[1][0]  # t coefficient
    p[RAW0 + 1] = pj[2][1] - pj[2][0]  # f coefficient
    p[RAW0 + 2] = e1c  # e coefficient
    p[RAW0 + 3] = e2c  # e^2 coefficient
    esel = np.zeros((2, VR), np.float16)
    esel[0, L0 : L0 + LN] = 1.0
    esel[1, P0 : P0 + PN] = 1.0
    iota = np.full((VR, 1), -1.0, np.float32)
    iota[L0 : L0 + LN, 0] = np.arange(LN)
    iota[P0 : P0 + PN, 0] = np.arange(PN)
    return {"ptab": p.astype(np.float16), "esel": esel, "iotac": iota}


def _make_in_maps(inputs, n_cores, n_core):
    shared = _make_consts(inputs)
    e2 = np.asarray(inputs["exchange_idx"], np.int32)
    e2 = (e2 * e2).astype(np.int32)
    in_maps = []
    for c in range(n_cores):
        m = dict(shared)
        for nm in IDX_NAMES:
            m[nm] = np.ascontiguousarray(
                np.asarray(inputs[nm], dtype=np.int32)[c * n_core : (c + 1) * n_core]
            )
        m["e2_idx"] = np.ascontiguousarray(e2[c * n_core : (c + 1) * n_core])
        in_maps.append(m)
    return in_maps


def run(inputs, trace=False, n_bcast=N_BCAST):
    """Run on hardware across 8 cores; returns (full_output, BassKernelResults)."""
    from concourse.bass_utils import run_bass_kernel_spmd

    n = np.asarray(inputs[IDX_NAMES[0]]).shape[0]
    n_core = n // N_CORES
    nc = _get_nc(n_core, n_bcast)
    in_maps = _make_in_maps(inputs, N_CORES, n_core)
    res = run_bass_kernel_spmd(nc, in_maps, core_ids=list(range(N_CORES)),
                               trace=trace)
    out = np.concatenate([r["y"] for r in res.results], axis=0)
    return out.astype(np.float32, copy=False), res


def kernel(**inputs):
    out, _ = run(inputs)
    return out


# revision 14
# speedup vs baseline: 1.0424x; 1.0424x over previous
"""Trainium2 Bass kernel for CompoundMultivariateEmbedding.

Math: out[n] = concat(level_tab[l], type_tab[t], feat_tab[f], exch_tab[e],
pair_tab[p]) @ W.T + b = P_lvl[l] + P_pair[p] + C0 + t*D1 + f*D2 + e*E1 + e^2*E2
where P_b = tab_b @ W[:, block_b].T.  The 2-row type/feature blocks are linear
in their index and the 3-row exchange block is an exact quadratic, so only
level (50 rows) and pair (20 rows) need one-hot treatment.  C0 (the constant
term + bias) folds into every level row.  The [88, 128] coefficient matrix P
is precomputed host-side in fp16 (quantization error ~5e-3 abs vs a ~0.11 abs
gate); e^2 is precomputed host-side as a sixth int32 index array.

Per-core loop (tokens sharded 8 ways, 131072/core, superbatches of 8192):
  1. six idx int32 -> fp16 SWDGE cast-DMAs (l -> p96, p -> p64 of idxf;
     t/f/e/e2 -> rows 84-87 of the stationary tile st directly)
  2. build idxb rows 0-49 (level) + 64-83 (pair): GPSIMD partition_broadcast
     (32-aligned bases 0/32/64) on bcast superbatches, or a tiny PE matmul
     (e_sel [2,84] stationary at rows 96-97, fp16 PSUM out) on the rest
  3. DVE is_equal vs per-partition iota -> one-hot st[0:84] fp16
     (4x mode from SBUF, 2x mode from PSUM); junk rows 50-63 hit P rows = 0
  4. per 1024 tokens: 8 matmuls, stationary = strided st[0:88] slice
     (token 8p+m -> partition p), moving = P fp16 [88, 128], fp32 PSUM out
  5. ACT copies PSUM -> SBUF staging; HWDGE stores 1 MiB per 2048 tokens
     (two 4 KiB contiguous chunks per partition)
"""

import sys

sys.path.insert(0, "/opt/trn_rl_repo")

import numpy as np

import concourse.bass as bass
import concourse.tile as tile
from concourse import bacc, library_config, mybir
from concourse._compat import with_exitstack

F32 = mybir.dt.float32
F16 = mybir.dt.float16
BF16 = mybir.dt.bfloat16
I32 = mybir.dt.int32

N_FULL = 1048576
N_CORES = 8
EMBED = 128

IDX_NAMES = ["level_idx", "type_idx", "feature_idx", "exchange_idx", "pair_idx"]

VR = 88  # stationary rows: 0-49 level, 50-63 junk, 64-83 pair, 84-87 t/f/e/e2
L0, LN = 0, 50
P0, PN = 64, 20
RAW0 = 84  # t, f, e, e2 rows

SB = 8192  # tokens per superbatch (idx DMA + one-hot build granularity)
TB = 1024  # tokens per PSUM batch (8 matmul tiles)
N_BCAST = 0  # GPSIMD partition_broadcast measured 8% of roofline + wrong: off


@with_exitstack
def _emb_kernel(ctx, tc, y_ap, pt_ap, esel_ap, iota_ap, idxs, n_core, n_bcast):
    nc = tc.nc

    if n_bcast > 0:
        nc.gpsimd.load_library(library_config.proxy)

    const = ctx.enter_context(tc.tile_pool(name="const", bufs=1))

    # ---- constants (host-precomputed) ----
    pt = const.tile([VR, EMBED], F16)  # coefficient matrix
    nc.sync.dma_start(pt, pt_ap)
    iota_col = const.tile([VR, 1], F32)  # within-block row index per partition
    nc.sync.dma_start(iota_col, iota_ap)
    e_sel = const.tile([128, VR], F16)  # level/pair masks at partitions 96-97
    nc.gpsimd.dma_start(e_sel[96:98, :], esel_ap)

    # ---- pools ----
    idx_pool = ctx.enter_context(tc.tile_pool(name="idxp", bufs=2))
    idxb_pool = ctx.enter_context(tc.tile_pool(name="idxbp", bufs=2))
    st_pool = ctx.enter_context(tc.tile_pool(name="stp", bufs=2))
    out_pool = ctx.enter_context(tc.tile_pool(name="outp", bufs=3))
    pbc_pool = ctx.enter_context(
        tc.tile_pool(name="pbc", bufs=2, space=bass.MemorySpace.PSUM)
    )
    pout_pool = ctx.enter_context(
        tc.tile_pool(name="pout", bufs=4, space=bass.MemorySpace.PSUM)
    )

    n_sb = n_core // SB

    def emit_loads(si):
        """Prefetch superbatch si: idx cast-DMAs into fresh idxf/st tiles."""
        s0 = si * SB
        st = st_pool.tile([VR, SB], F16, tag="st")
        idxf = idx_pool.tile([128, SB], F16, tag="idxf")
        nc.gpsimd.dma_start(idxf[96:97, :], idxs[0][s0 : s0 + SB])
        nc.gpsimd.dma_start(idxf[97:98, :], idxs[4][s0 : s0 + SB])
        for k, j in enumerate([1, 2, 3, 5]):
            nc.gpsimd.dma_start(st[RAW0 + k : RAW0 + k + 1, :], idxs[j][s0 : s0 + SB])
        return idxf, st

    def emit_build(idxf, st, h2):
        """One-hot build for tokens [h2*1024, (h2+1)*1024) of a superbatch."""
        psbc = pbc_pool.tile([RAW0, 1024], F32, tag="psbc")
        for g in range(2):
            nc.tensor.matmul(
                psbc[:, bass.ts(g, 512)],
                e_sel[96:98, 0:RAW0],
                idxf[96:98, h2 * 1024 + g * 512 : h2 * 1024 + (g + 1) * 512],
                tile_position=(96, 0),
            )
        nc.vector.tensor_scalar(
            st[0:RAW0, bass.ts(h2, 1024)],
            psbc,
            iota_col[0:RAW0, :],
            None,
            mybir.AluOpType.is_equal,
        )

    def emit_store(si, st, tbp, build_units):
        """Project + store tokens [tbp*2048, (tbp+1)*2048) of superbatch si,
        interleaving next-superbatch one-hot builds between pso units."""
        st_r = st.rearrange("v (t p q) -> v t q p", q=8, p=128)
        osb = out_pool.tile([128, 2 * TB], F32, tag="osb")
        for u in range(4):  # 512-token pso units
            if u % 2 == 0 and build_units:
                build_units.pop(0)()
            tb = tbp * 2 + u // 2
            pso = pout_pool.tile([128, 512], F32, tag="pso")
            for m in range(4):
                mg = (u % 2) * 4 + m
                nc.tensor.matmul(
                    pso[:, bass.ts(m, 128)],
                    st_r[:, tb, mg, :],
                    pt,
                    start=True,
                    stop=True,
                )
            nc.scalar.copy(osb[:, u * 512 : (u + 1) * 512], pso)
        n0 = si * SB + tbp * 2 * TB
        dview = y_ap[n0 : n0 + 2 * TB, :].rearrange(
            "(h p q) e -> p h q e", h=2, p=128, q=8
        )
        nc.sync.dma_start(dview, osb.rearrange("p (h q e) -> p h q e", h=2, q=8))

    # software pipeline: build superbatch sb+1 interleaved with sb's stores
    cur = emit_loads(0)
    for h2 in range(SB // 1024):
        emit_build(cur[0], cur[1], h2)
    for sb in range(n_sb):
        nxt = emit_loads(sb + 1) if sb + 1 < n_sb else None
        builds = (
            [
                (lambda h2=h2: emit_build(nxt[0], nxt[1], h2))
                for h2 in range(SB // 1024)
            ]
            if nxt is not None
            else []
        )
        for tbp in range(SB // (2 * TB)):
            emit_store(sb, cur[1], tbp, builds)
        cur = nxt

def build(n_core, n_bcast=N_BCAST, num_devices=N_CORES):
    nc = bacc.Bacc(
        "TRN2", target_bir_lowering=False, debug=False, num_devices=num_devices
    )
    pt_ap = nc.dram_tensor("ptab", [VR, EMBED], F16, kind="ExternalInput").ap()
    esel_ap = nc.dram_tensor("esel", [2, VR], F16, kind="ExternalInput").ap()
    iota_ap = nc.dram_tensor("iotac", [VR, 1], F32, kind="ExternalInput").ap()
    idxs = []
    for nm in IDX_NAMES + ["e2_idx"]:
        idxs.append(nc.dram_tensor(nm, [n_core], I32, kind="ExternalInput").ap())
    y = nc.dram_tensor("y", [n_core, EMBED], F32, kind="ExternalOutput")

    with tile.TileContext(nc) as tc:
        _emb_kernel(tc, y.ap(), pt_ap, esel_ap, iota_ap, idxs, n_core, n_bcast)
    nc.compile()
    return nc


_NC_CACHE = {}


def _get_nc(n_core, n_bcast=N_BCAST):
    key = (n_core, n_bcast)
    if key not in _NC_CACHE:
        _NC_CACHE[key] = build(n_core, n_bcast)
    return _NC_CACHE[key]


def _make_consts(inputs):
    """Host-side prep of the tiny [88,128] fp16 coefficient matrix + masks."""
    w = np.asarray(inputs["W"], np.float32)
    b = np.asarray(inputs["b"], np.float32)
    tabs = [
        np.asarray(inputs[nm], np.float32)
        for nm in ["level_tab", "type_tab", "feature_tab", "exchange_tab", "pair_tab"]
    ]
    foff = [0, 25, 50, 75, 100]
    pj = [t @ w[:, f : f + t.shape[1]].T for t, f in zip(tabs, foff)]
    c0 = pj[1][0] + pj[2][0] + pj[3][0] + b  # type/feat/exch row 0 + bias
    e2c = (pj[3][2] - 2.0 * pj[3][1] + pj[3][0]) / 2.0
    e1c = pj[3][1] - pj[3][0] - e2c
    p = np.zeros((VR, EMBED), np.float32)
    p[L0 : L0 + LN] = pj[0] + c0[None, :]
    p[P0 : P0 + PN] = pj[4]
    p[RAW0 + 0] = pj[1][1] - pj[1][0]  # t coefficient
    p[RAW0 + 1] = pj[2][1] - pj[2][0]  # f coefficient
    p[RAW0 + 2] = e1c  # e coefficient
    p[RAW0 + 3] = e2c  # e^2 coefficient
    esel = np.zeros((2, VR), np.float16)
    esel[0, L0 : L0 + LN] = 1.0
    esel[1, P0 : P0 + PN] = 1.0
    iota = np.full((VR, 1), -1.0, np.float32)
    iota[L0 : L0 + LN, 0] = np.arange(LN)
    iota[P0 : P0 + PN, 0] = np.arange(PN)
    return {"ptab": p.astype(np.float16), "esel": esel, "iotac": iota}


def _make_in_maps(inputs, n_cores, n_core):
    shared = _make_consts(inputs)
    e2 = np.asarray(inputs["exchange_idx"], np.int32)
    e2 = (e2 * e2).astype(np.int32)
    in_maps = []
    for c in range(n_cores):
        m = dict(shared)
        for nm in IDX_NAMES:
            m[nm] = np.ascontiguousarray(
                np.asarray(inputs[nm], dtype=np.int32)[c * n_core : (c + 1) * n_core]
            )
        m["e2_idx"] = np.ascontiguousarray(e2[c * n_core : (c + 1) * n_core])
        in_maps.append(m)
    return in_maps


def run(inputs, trace=False, n_bcast=N_BCAST):
    """Run on hardware across 8 cores; returns (full_output, BassKernelResults)."""
    from concourse.bass_utils import run_bass_kernel_spmd

    n = np.asarray(inputs[IDX_NAMES[0]]).shape[0]
    n_core = n // N_CORES
    nc = _get_nc(n_core, n_bcast)
    in_maps = _make_in_maps(inputs, N_CORES, n_core)
    res = run_bass_kernel_spmd(nc, in_maps, core_ids=list(range(N_CORES)),
                               trace=trace)
    out = np.concatenate([r["y"] for r in res.results], axis=0)
    return out.astype(np.float32, copy=False), res


def kernel(**inputs):
    out, _ = run(inputs)
    return out
